# revision 29
# baseline (speedup 1.0000x reference)
"""Trainium2 Bass kernel for packed-varlen causal attention (16 heads, D=1024).

Strategy: data-parallel over segments across 8 NeuronCores. Each core packs
1-2 segments tile-aligned into a 1536-token buffer. One SPMD program; all
per-core differences are data (packed inputs + 0/1 masks).

v2: engine-balanced pipeline.
- RoPE consumes projection PSUM directly on DVE (partition-shifted reads do
  rotate-half; sign baked into sinT host-side). No ACT copies, no DMA swaps.
- Attention is software-pipelined: scores for entry ik+1 are emitted before
  the PV matmuls of entry ik so the PE queue never head-of-line blocks.
- Softmax denominators: per head-pair [2,512] reciprocal_approx_fast (DVE),
  broadcast to 128 partitions with a single K=2 matmul against a 0/1
  selector, normalization fused into the PSUM->SBUF evacuation multiply.
- Masks are loaded/applied only where some core's mask is not all-ones
  (host-side union analysis); applied in-place on sub-spans.
- All PSUM evacuations on DVE; output DMA'd as bf16 and upcast on host.
"""
import os
from contextlib import ExitStack

import numpy as np
import ml_dtypes

import concourse.bass as bass
import concourse.tile as tile
from concourse import bacc, mybir
from concourse.bass_utils import run_bass_kernel_spmd

BF16 = ml_dtypes.bfloat16
F32 = np.float32
NCORES = 8
NT = 12            # query tiles of 128 -> 1536 token slots per core
TOK = NT * 128
EMBED, HEADS, HDIM = 1024, 16, 64
DT = mybir.dt

LAST_EXEC_NS = None
LAST_TRACE = None
_CACHE = {}


def _install_ntff_shim():
    """Provide antenv.axon_hooks (missing in this image) so
    run_bass_kernel_spmd(trace=True) can capture NTFF profiles via the
    axon .so, and keep artifacts local instead of uploading."""
    import sys
    import types
    try:
        import antenv.axon_hooks  # noqa: F401
        return
    except ImportError:
        pass
    try:
        from trn_agent_boot.trn_boot import _ntff_profile_via_ctypes
        hook = _ntff_profile_via_ctypes("/opt/axon/libaxon_pjrt.so")
    except Exception:
        hook = None
    mod = types.ModuleType("antenv.axon_hooks")
    mod.get_axon_ntff_profile_hook = lambda: hook
    mod.set_axon_ntff_profile_hook = lambda h: None
    sys.modules["antenv.axon_hooks"] = mod
    import concourse.bass_utils as _bu
    _bu.upload_artifacts = lambda tmpdir: tmpdir


# ---------------------------------------------------------------- planning --

def _build_plan(seq_lens):
    segs = sorted(range(len(seq_lens)), key=lambda i: -int(seq_lens[i]))
    loads = [0.0] * NCORES
    tiles_used = [0] * NCORES
    assign = [[] for _ in range(NCORES)]
    for s in segs:
        L = int(seq_lens[s])
        nt = (L + 127) // 128
        cost = L * 8.4e6 + (L * L) * 2048.0
        placed = False
        for c in sorted(range(NCORES), key=lambda c: loads[c]):
            if tiles_used[c] + nt <= NT:
                assign[c].append(s)
                loads[c] += cost
                tiles_used[c] += nt
                placed = True
                break
        assert placed, "segments do not fit the 8x1536 structure"
    core_chunks = []
    for c in range(NCORES):
        t0, chunks = 0, []
        for s in assign[c]:
            L = int(seq_lens[s])
            chunks.append((s, t0, L))
            t0 += (L + 127) // 128
        core_chunks.append(chunks)
    return core_chunks


def _build_structure(core_chunks, seq_lens):
    """Union causal structure + per-entry mask requirements.

    Per core, token slot t belongs to segment s with in-segment position p
    (or is padding). A (k-tile kj, q-tile qi) pair is computed if any core
    has same-segment causal overlap there. A mask is applied on the sub-span
    of q-tiles where any core's {same-seg & causal & valid (incl. diag)}
    mask tile is not all-ones.
    """
    segid = np.full((NCORES, TOK), -1, np.int64)
    pos = np.zeros((NCORES, TOK), np.int64)
    for c in range(NCORES):
        for (s, t0, L) in core_chunks[c]:
            sl = slice(t0 * 128, t0 * 128 + L)
            segid[c, sl] = s
            pos[c, sl] = np.arange(L)

    compute = np.zeros((NT, NT), bool)      # [qi, kj]
    allones = np.ones((NCORES, NT, NT), bool)
    for c in range(NCORES):
        same = (segid[c][:, None] == segid[c][None, :]) & (segid[c][:, None] >= 0)
        # m01[k, q] = same-seg & (pos_k <= pos_q), plus forced diagonal
        causal = pos[c][:, None] <= pos[c][None, :]
        m01 = (same & causal) | np.eye(TOK, dtype=bool)
        for qi in range(NT):
            qs = slice(qi * 128, qi * 128 + 128)
            for kj in range(qi + 1):
                ks = slice(kj * 128, kj * 128 + 128)
                # pair computed if this core has real overlap (any same-seg)
                if (same[qs, ks]).any():
                    compute[qi, kj] = True
                allones[c, qi, kj] = bool(m01[ks, qs].all())
    nonones = compute & (~allones).any(axis=0)
    # every computed pair: mask needed where any core tile is not all-ones
    structure = []
    for cch in range(3):
        qlo_c, qhi_c = 4 * cch, 4 * cch + 4
        klist = []
        for kj in range(NT):
            qs = [qi for qi in range(qlo_c, qhi_c) if compute[qi, kj]]
            if not qs:
                continue
            qlo, qhi = min(qs), max(qs) + 1
            mqs = [qi for qi in range(qlo, qhi) if nonones[qi, kj]]
            if mqs:
                mlo, mhi = min(mqs), max(mqs) + 1
            else:
                mlo, mhi = 0, 0
            klist.append((kj, qlo, qhi, mlo, mhi))
        structure.append(klist)
    return structure


# ---------------------------------------------------------- device program --

def _emit_program(structure):
    nc = bacc.Bacc("TRN2", target_bir_lowering=False, debug=False,
                   num_devices=NCORES)
    f32, bf16 = DT.float32, DT.bfloat16
    EXP = mybir.ActivationFunctionType.Exp

    xT_d = nc.dram_tensor("xT", [EMBED, TOK], bf16, kind="ExternalInput").ap()
    cosT_d = nc.dram_tensor("cosT", [128, TOK], f32, kind="ExternalInput").ap()
    sinT_d = nc.dram_tensor("sinT", [128, TOK], f32, kind="ExternalInput").ap()
    maskT_d = nc.dram_tensor("maskT", [TOK, TOK], bf16, kind="ExternalInput").ap()
    wq_d = nc.dram_tensor("wqT", [EMBED, EMBED], bf16, kind="ExternalInput").ap()
    wk_d = nc.dram_tensor("wkT", [EMBED, EMBED], bf16, kind="ExternalInput").ap()
    wv_d = nc.dram_tensor("wvT", [EMBED, EMBED], bf16, kind="ExternalInput").ap()
    wo_d = nc.dram_tensor("woT", [EMBED, EMBED], bf16, kind="ExternalInput").ap()
    qb_d = nc.dram_tensor("qb", [1, EMBED], bf16, kind="ExternalInput").ap()
    vb_d = nc.dram_tensor("vb", [1, EMBED], bf16, kind="ExternalInput").ap()
    ob_d = nc.dram_tensor("ob", [1, EMBED], bf16, kind="ExternalInput").ap()
    yT_d = nc.dram_tensor("yT", [EMBED, TOK], bf16, kind="ExternalOutput").ap()

    xr_d = xT_d.rearrange("(a p) t -> p a t", p=128)

    with tile.TileContext(nc) as tc, ExitStack() as ctx:
        singles = ctx.enter_context(tc.tile_pool(name="singles", bufs=1))
        wopool = ctx.enter_context(tc.tile_pool(name="wopool", bufs=1))
        persist = ctx.enter_context(tc.tile_pool(name="persist", bufs=1))
        qrpool = ctx.enter_context(tc.tile_pool(name="qrpool", bufs=2))
        wpool = ctx.enter_context(tc.tile_pool(name="wpool", bufs=2))
        xpool = ctx.enter_context(tc.tile_pool(name="xpool", bufs=2))
        cspool = ctx.enter_context(tc.tile_pool(name="cspool", bufs=1))
        rope = ctx.enter_context(tc.tile_pool(name="rope", bufs=2))
        mpool = ctx.enter_context(tc.tile_pool(name="mpool", bufs=1))
        epool = ctx.enter_context(tc.tile_pool(name="epool", bufs=2))
        rpool = ctx.enter_context(tc.tile_pool(name="rpool", bufs=1))
        aunp = ctx.enter_context(tc.tile_pool(name="aunp", bufs=2))
        attnp = ctx.enter_context(tc.tile_pool(name="attnp", bufs=2))
        ypool = ctx.enter_context(tc.tile_pool(name="ypool", bufs=1))
        # PSUM (8 banks): acc 2 + s (2 banks x 2 bufs) 4 + pa 2
        accp = ctx.enter_context(tc.tile_pool(name="accp", bufs=2, space="PSUM"))
        spool = ctx.enter_context(tc.tile_pool(name="spool", bufs=2, space="PSUM"))
        papool = ctx.enter_context(tc.tile_pool(name="papool", bufs=1, space="PSUM"))

        # constants / persistent tensors
        qb_sb = singles.tile([1, EMBED], bf16, tag="qb")
        nc.sync.dma_start(out=qb_sb, in_=qb_d)
        vb_sb = singles.tile([1, EMBED], bf16, tag="vb")
        nc.sync.dma_start(out=vb_sb, in_=vb_d)
        ob_sb = singles.tile([1, EMBED], bf16, tag="ob")
        nc.sync.dma_start(out=ob_sb, in_=ob_d)
        ones_sb = singles.tile([1, 512], bf16, tag="ones")
        nc.vector.memset(ones_sb, 1.0)
        # selector rows replicated at each 32-aligned slot (matmul requires
        # lhsT and rhs to share a base partition)
        sel0_sb = singles.tile([97, 128], bf16, tag="sel0")
        nc.vector.memset(sel0_sb, 0.0)
        sel1_sb = singles.tile([97, 128], bf16, tag="sel1")
        nc.vector.memset(sel1_sb, 0.0)
        for b in (0, 32, 64, 96):
            nc.vector.memset(sel0_sb[b:b + 1, 0:64], 1.0)
            nc.vector.memset(sel1_sb[b:b + 1, 64:128], 1.0)

        qbT_sb = singles.tile([128, 8], bf16, tag="qbT")
        nc.sync.dma_start(out=qbT_sb,
                          in_=qb_d.rearrange("o (m p) -> p (o m)", p=128))
        vbT_sb = singles.tile([128, 8], bf16, tag="vbT")
        nc.sync.dma_start(out=vbT_sb,
                          in_=vb_d.rearrange("o (m p) -> p (o m)", p=128))
        # HAM pre-warm: dummy matmuls during the initial DMA wait keep the
        # PE activity monitor at full clock before real work lands
        warm = accp.tile([128, 512], f32, tag="acc")
        for _ in range(40):
            nc.tensor.matmul(warm, sel0_sb[0:1, :], ones_sb, start=True,
                             stop=True)

        kr_sb = persist.tile([128, 8, TOK], bf16, tag="kr")
        # v with a ones column appended per head: [tok_tile, head, 65]
        va_sb = persist.tile([128, NT, HEADS, HDIM + 1], bf16, tag="va")
        nc.vector.memset(va_sb[:, :, :, HDIM:HDIM + 1], 1.0)

        def load_w(dram, pool, tag):
            w = pool.tile([128, 8, EMBED], bf16, tag=tag)
            dr = dram.rearrange("(a p) n -> p a n", p=128)
            for a in range(8):
                nc.sync.dma_start(out=w[:, a, :], in_=dr[:, a, :])
            return w

        def load_x(c3):
            x = xpool.tile([128, 8, 512], bf16, tag="x")
            for a in range(8):
                nc.sync.dma_start(out=x[:, a, :],
                                  in_=xr_d[:, a, bass.ts(c3, 512)])
            return x

        # --------------------------------------------------- V projection --
        wv = load_w(wv_d, wpool, "w")
        for c3 in range(3):
            x_sb = load_x(c3)
            for t4 in range(4):
                tt = 4 * c3 + t4
                for n2 in range(2):
                    ps = accp.tile([128, 512], f32, tag="acc")
                    for a in range(8):
                        nc.tensor.matmul(ps, x_sb[:, a, bass.ts(t4, 128)],
                                         wv[:, a, bass.ts(n2, 512)],
                                         start=(a == 0), stop=False)
                    nc.tensor.matmul(ps, ones_sb[:, 0:128],
                                     vb_sb[:, bass.ts(n2, 512)], start=False,
                                     stop=True)
                    nc.scalar.copy(va_sb[:, tt, bass.ts(n2, 8), 0:HDIM],
                                   ps.rearrange("p (h d) -> p h d", d=HDIM))
        wq = load_w(wq_d, wpool, "w")
        wk = load_w(wk_d, wpool, "w")
        wo_sb = load_w(wo_d, wopool, "wo")

        def rope_proj(w_sb, bias_row, out_view, m, x_sb, cos_sb, sin_sb):
            # out_view[:, m, :] <- rope(proj) for one 512-token chunk
            if True:
                ps = accp.tile([128, 512], f32, tag="acc")
                for a in range(8):
                    nc.tensor.matmul(ps, w_sb[:, a, bass.ts(m, 128)],
                                     x_sb[:, a, :], start=(a == 0),
                                     stop=(a == 7))
                # ACT evacuates PSUM once (folding in the per-feature
                # bias); rotate-half shuffle via SBUF->SBUF DMA
                # (engine-free); SBUF-only DVE math
                qc = rope.tile([128, 512], bf16, tag="qc")
                if bias_row is not None:
                    nc.scalar.activation(
                        qc, ps, mybir.ActivationFunctionType.Identity,
                        bias=bias_row[:, m:m + 1])
                else:
                    nc.scalar.copy(qc, ps)
                sw = rope.tile([128, 512], bf16, tag="sw")
                for eng, b in ((nc.sync, 0), (nc.scalar, 64)):
                    eng.dma_start(out=sw[b:b + 32, :],
                                  in_=qc[b + 32:b + 64, :])
                    eng.dma_start(out=sw[b + 32:b + 64, :],
                                  in_=qc[b:b + 32, :])
                m1 = rope.tile([128, 512], bf16, tag="m1")
                nc.vector.tensor_mul(m1, qc, cos_sb)
                m2 = rope.tile([128, 512], bf16, tag="m2")
                nc.vector.tensor_mul(m2, sw, sin_sb)
                nc.vector.tensor_add(out_view[:, m, :], m1, m2)

        # ------------------------- per 512-token chunk: rope + attention --
        kr_view = kr_sb.rearrange("p a (c t) -> p a c t", t=512)

        def chunk_rope_groups(c3):
            """List of per-m-group emitters for ropeQ+ropeK of chunk c3."""
            t5 = bass.ts(c3, 512)
            x_sb = load_x(c3)
            cos_sb = cspool.tile([128, 512], f32, tag="cos")
            nc.scalar.dma_start(out=cos_sb, in_=cosT_d[:, t5])
            sin_sb = cspool.tile([128, 512], f32, tag="sin")
            nc.scalar.dma_start(out=sin_sb, in_=sinT_d[:, t5])
            qr_c = qrpool.tile([128, 8, 512], bf16, tag="qr")
            kr_c = kr_view[:, :, c3, :].rearrange("p a t -> p a t")
            groups = []
            for m in range(8):
                groups.append((wq, qbT_sb, qr_c, m, x_sb, cos_sb, sin_sb))
                groups.append((wk, None, kr_c, m, x_sb, cos_sb, sin_sb))
            return qr_c, groups

        qr0, groups0 = chunk_rope_groups(0)
        for g in groups0:
            rope_proj(*g)
        qr_next = qr0
        pending = []
        for cch in range(3):
            q0 = cch * 512
            qr_c = qr_next
            if cch < 2:
                qr_next, groups_next = chunk_rope_groups(cch + 1)
            else:
                groups_next = []

            # ----------------------------------------------- attention --
            klist = structure[cch]
            mts = {}
            for ix, (kj, qlo, qhi, mlo, mhi) in enumerate(klist):
                if mhi > mlo:
                    nm = (mhi - mlo) * 128
                    mt = mpool.tile([128, 512], bf16, tag=f"m{ix % 8}")
                    nc.sync.dma_start(
                        out=mt[:, 0:nm],
                        in_=maskT_d[bass.ts(kj, 128), bass.ds(mlo * 128, nm)])
                    mts[kj] = mt
            aun = aunp.tile([128, 8, 512], bf16, tag="aun")
            attn_sb = attnp.tile([128, 8, 512], bf16, tag="attn")
            den_g = [rpool.tile([97, 2, 512], f32, tag=f"den{g}",
                                name=f"den{g}")
                     for g in range(2)]
            nc.gpsimd.memset(den_g[0], 1.0)
            nc.gpsimd.memset(den_g[1], 1.0)
            rec_g = [None, None]

            def normalize(hpx, rec_g=rec_g, aun=aun, attn_sb=attn_sb):
                recb = rec_g[hpx // 4]
                b = 32 * (hpx % 4)
                rb = accp.tile([128, 512], f32, tag="acc")
                nc.tensor.matmul(rb, sel0_sb[b:b + 1, :],
                                 recb[b:b + 1, 0, :], start=True,
                                 stop=False, tile_position=(b, 0))
                nc.tensor.matmul(rb, sel1_sb[b:b + 1, :],
                                 recb[b:b + 1, 1, :], start=False,
                                 stop=True, tile_position=(b, 0))
                nc.vector.tensor_mul(attn_sb[:, hpx, :], aun[:, hpx, :], rb)

            for hp in range(8):
                pa = papool.tile([HDIM + 1, 2, 512], f32, tag="pa")

                def consume(s, ent, ik):
                    kj, qlo, qhi, mlo, mhi = ent
                    nq = (qhi - qlo) * 128
                    qoff = qlo * 128 - q0
                    e = epool.tile([128, 2, 512], bf16, tag="e")
                    nc.scalar.activation(e[:, :, 0:nq], s[:, :, 0:nq],
                                         EXP, scale=0.125)
                    if mhi > mlo:
                        moff = (mlo - qlo) * 128
                        nm = (mhi - mlo) * 128
                        mb = mts[kj][:, 0:nm].rearrange(
                            "p (o n) -> p o n", o=1).broadcast_to(
                            [128, 2, nm])
                        esl = e[:, :, bass.ds(moff, nm)]
                        nc.vector.tensor_mul(esl, esl, mb)
                    for i in range(2):
                        h = 2 * hp + i
                        nc.tensor.matmul(
                            pa[:, i, bass.ds(qoff, nq)],
                            va_sb[:, kj, h, :], e[:, i, 0:nq],
                            start=(ik == 0), stop=(ik == len(klist) - 1),
                            skip_group_check=True)

                prev = None
                for ik, ent in enumerate(klist):
                    kj, qlo, qhi, mlo, mhi = ent
                    nq = (qhi - qlo) * 128
                    s = spool.tile([128, 2, 512], f32, tag="s")
                    for i in range(2):
                        h = 2 * hp + i
                        krs = kr_sb[bass.ds((h % 2) * 64, 64), h // 2,
                                    bass.ts(kj, 128)]
                        qrs = qr_c[bass.ds((h % 2) * 64, 64), h // 2,
                                   bass.ds(qlo * 128 - q0, nq)]
                        nc.tensor.matmul(s[:, i, 0:nq], krs, qrs,
                                         start=True, stop=True)
                    if prev is not None:
                        consume(*prev)
                    prev = (s, ent, ik)
                consume(*prev)
                for g in groups_next[2 * hp:2 * hp + 2]:
                    rope_proj(*g)
                _starts = (0, 2, 3, 5, 6, 8, 10, 11)
                _ends = (2, 3, 5, 6, 8, 10, 11, 12)
                for it in pending[_starts[hp]:_ends[hp]]:
                    it()

                # epilogue: stash denominators + unnormalized values;
                # reciprocal + normalization pipelined 4 hp behind
                g, b = hp // 4, 32 * (hp % 4)
                nc.scalar.copy(den_g[g][b:b + 1, :, :], pa[64:65, :, :])
                nc.vector.tensor_copy(aun[0:64, hp, :], pa[0:64, 0, :])
                nc.vector.tensor_copy(aun[64:128, hp, :], pa[0:64, 1, :])
                if hp % 4 == 3:
                    nc.vector.reciprocal_approx_fast(out=den_g[g],
                                                     in_=den_g[g])
                    recb = rpool.tile([97, 2, 512], bf16, tag=f"recb{g}")
                    nc.vector.tensor_copy(recb, den_g[g])
                    rec_g[g] = recb
                if hp >= 4:
                    normalize(hp - 4)
            # leftover pending items from the previous chunk
            for it in pending[12:]:
                it()

            def mk_norm(hpx, normalize=normalize):
                return lambda: normalize(hpx)

            def mk_outproj(m, cch=cch, attn_sb=attn_sb):
                def f():
                    py = accp.tile([128, 512], f32, tag="acc")
                    for r in range(8):
                        nc.tensor.matmul(py, wo_sb[:, r, bass.ts(m, 128)],
                                         attn_sb[:, r, :], start=(r == 0),
                                         stop=False)
                    nc.tensor.matmul(py, ob_sb[:, bass.ts(m, 128)],
                                     ones_sb, start=False, stop=True)
                    ys = ypool.tile([128, 512], bf16, tag="ys")
                    nc.vector.tensor_copy(ys, py)
                    nc.sync.dma_start(
                        out=yT_d[bass.ts(m, 128), bass.ts(cch, 512)], in_=ys)
                return f

            pending = ([mk_norm(hpx) for hpx in range(4, 8)] +
                       [mk_outproj(m) for m in range(8)])
        for it in pending:
            it()
    nc.compile()
    return nc


# ------------------------------------------------------------- host driver --

def _host_prep(hidden, cos, sin, seq_lens, core_chunks):
    starts = np.concatenate([[0], np.cumsum(seq_lens)]).astype(np.int64)
    per_core = []
    sgn = np.concatenate([-np.ones(32, F32), np.ones(32, F32)])
    for c in range(NCORES):
        tokmap = np.full(TOK, -1, np.int64)
        segid = np.full(TOK, -1, np.int64)
        pos = np.zeros(TOK, np.int64)
        for (s, t0, L) in core_chunks[c]:
            sl = slice(t0 * 128, t0 * 128 + L)
            tokmap[sl] = np.arange(starts[s], starts[s] + L)
            segid[sl] = s
            pos[sl] = np.arange(L)
        real = tokmap >= 0
        x = np.zeros((TOK, EMBED), F32)
        x[real] = hidden[tokmap[real]]
        cs = np.zeros((TOK, HDIM), F32)
        sn = np.zeros((TOK, HDIM), F32)
        cs[real] = cos[tokmap[real]]
        sn[real] = sin[tokmap[real]]
        cosT = np.tile(np.ascontiguousarray(cs.T), (2, 1)).astype(F32)
        sinT = np.tile(np.ascontiguousarray(sn.T) * sgn[:, None],
                       (2, 1)).astype(F32)
        same = (segid[:, None] == segid[None, :]) & (segid[:, None] >= 0)
        causal = pos[:, None] <= pos[None, :]
        m01 = (same & causal) | np.eye(TOK, dtype=bool)
        maskT = m01.astype(BF16)
        per_core.append(dict(tokmap=tokmap,
                             xT=np.ascontiguousarray(x.T).astype(BF16),
                             cosT=cosT, sinT=sinT, maskT=maskT))
    return per_core


def kernel(hidden_states, cos, sin, q_w, q_b, k_w, v_w, v_b, out_w, out_b,
           seq_len, max_seqlen):
    global LAST_EXEC_NS
    hidden = np.asarray(hidden_states, F32)
    cos = np.asarray(cos, F32)
    sin = np.asarray(sin, F32)
    seq_lens = [int(v) for v in np.asarray(seq_len)]

    core_chunks = _build_plan(seq_lens)
    structure = _build_structure(core_chunks, seq_lens)
    key = tuple(tuple(map(tuple, s)) for s in structure)
    if key not in _CACHE:
        _CACHE[key] = _emit_program(structure)
    nc = _CACHE[key]

    per_core = _host_prep(hidden, cos, sin, seq_lens, core_chunks)
    shared = {
        "wqT": np.ascontiguousarray(np.asarray(q_w, F32).T).astype(BF16),
        "wkT": np.ascontiguousarray(np.asarray(k_w, F32).T).astype(BF16),
        "wvT": np.ascontiguousarray(np.asarray(v_w, F32).T).astype(BF16),
        "woT": np.ascontiguousarray(np.asarray(out_w, F32).T).astype(BF16),
        "qb": np.asarray(q_b, F32).reshape(1, EMBED).astype(BF16),
        "vb": np.asarray(v_b, F32).reshape(1, EMBED).astype(BF16),
        "ob": np.asarray(out_b, F32).reshape(1, EMBED).astype(BF16),
    }
    in_maps = []
    for c in range(NCORES):
        pc = per_core[c]
        in_maps.append({**shared, "xT": pc["xT"], "cosT": pc["cosT"],
                        "sinT": pc["sinT"], "maskT": pc["maskT"]})

    trace = os.environ.get("BASS_KERNEL_TRACE", "0") == "1"
    if trace:
        _install_ntff_shim()
    import time as _time
    _t0 = _time.time()
    res = run_bass_kernel_spmd(nc, in_maps, core_ids=list(range(NCORES)),
                               trace=trace)
    LAST_EXEC_NS = res.exec_time_ns
    globals()["LAST_TRACE"] = res.instructions_and_trace
    globals()["LAST_RUN_WALL_S"] = _time.time() - _t0

    T = hidden.shape[0]
    out = np.zeros((T, EMBED), F32)
    for c in range(NCORES):
        tokmap = per_core[c]["tokmap"]
        real = tokmap >= 0
        yT = np.asarray(res.results[c]["yT"], F32)
        out[tokmap[real]] = yT.T[real]
    return out
